# revision 6
# baseline (speedup 1.0000x reference)
"""Causal self-attention (B=4, T=2048, C=1024, H=16) on 8 trn2 NeuronCores.

Sharding: core = (batch b, head-group g), b in 0..3, g in 0..1. Each core does
8 heads of one batch element (Megatron column split of w_attn, row split of
w_proj); host sums the two partial projection outputs per batch element.

Per-core kernel, v4 (eager-start, globally software-pipelined, PE kept dense):
 - All DRAM inputs bf16; output bf16 (host upcasts and sums partials).
 - Q^T,K^T computed transposed (lhsT=W-block, rhs=x^T-block) so attention
   needs no transposes; V natural with a ones column per head so the
   attention AV matmul accumulates the softmax denominator l for free.
 - Eager start: only V token-blocks 0-3 and the first qt/kt chunk are
   computed before attention begins; the remaining V blocks and qt/kt
   chunks become PE filler units pumped just-in-time inside the
   ACT-bound attention loop (measured exp rate ~1.16 ns/elem makes the
   steady state a knife-edge PE/ACT balance, so fillers are spread
   evenly: every k-block for hp 0/3, every 4th for hp 1/2).
 - Attention per head-pair: S^T for both heads row-tiled into one
   [128,1024] PSUM tile per k-block; one exp per k-block (3D AP covers
   both heads, scale=1/8 folded in); causal mask only on diagonal
   blocks via one doubled-mask bf16 multiply; AV deferred DEPTH k-blocks
   through a GLOBAL queue crossing qc/head-pair boundaries so the
   S->exp->AV pipeline never drains mid-kernel (keeps PE p-state at max
   clock).
 - qc finalize inline (attached to the last deferred AV): Y^T copy on
   the scalar engine (Copy activation), reciprocal straight from PSUM
   on vector, rank-1 broadcast matmul + in-place multiply immediately;
   output projection for the finished token blocks queued as filler.
 - Startup: DMA triggers spread across engine queues (sync: wv/mask/wp,
   gpsimd: x halves, scalar: hp0 qk weights) so descriptor writes don't
   serialize.
"""

import sys

if "/opt/trn_rl_repo" not in sys.path:
    sys.path.insert(0, "/opt/trn_rl_repo")

import numpy as np

T = 2048
C = 1024
G = 512          # per-core head-group width (8 heads x 64)
D = 64           # head dim
NH = 8           # heads per core
QCH = 512        # query chunk
KBLK = 128       # key block
DEPTH = 3        # AV deferral depth in k-blocks (global queue)


def _build_nc():
    from collections import deque
    from contextlib import ExitStack

    import concourse.bass as bass
    import concourse.mybir as mybir
    import concourse.tile as tile
    from concourse import bacc

    F32 = mybir.dt.float32
    F32R = mybir.dt.float32r
    BF16 = mybir.dt.bfloat16
    EXP = mybir.ActivationFunctionType.Exp
    CPY = mybir.ActivationFunctionType.Copy

    nc = bacc.Bacc("TRN2", target_bir_lowering=False)

    xT = nc.dram_tensor("xT", [C, T], BF16, kind="ExternalInput")
    wq = nc.dram_tensor("wq", [C, G], BF16, kind="ExternalInput")
    wk = nc.dram_tensor("wk", [C, G], BF16, kind="ExternalInput")
    wv = nc.dram_tensor("wv", [C, G], BF16, kind="ExternalInput")
    wp = nc.dram_tensor("wp", [G, C], BF16, kind="ExternalInput")
    mask = nc.dram_tensor("mask", [128, 256], BF16, kind="ExternalInput")
    out = nc.dram_tensor("out", [T, C], BF16, kind="ExternalOutput")

    with tile.TileContext(nc) as tc, ExitStack() as ctx:
        persist = ctx.enter_context(tc.tile_pool(name="persist", bufs=1))
        xw = ctx.enter_context(tc.tile_pool(name="xw", bufs=1))
        wsl = ctx.enter_context(tc.tile_pool(name="wsl", bufs=1))
        wqk = ctx.enter_context(tc.tile_pool(name="wqk", bufs=2))
        qtkt = ctx.enter_context(tc.tile_pool(name="qtkt", bufs=2))
        ptp = ctx.enter_context(tc.tile_pool(name="ptp", bufs=DEPTH + 1))
        nrm = ctx.enter_context(tc.tile_pool(name="nrm", bufs=2))
        lrp = ctx.enter_context(tc.tile_pool(name="lrp", bufs=4))
        osb = ctx.enter_context(tc.tile_pool(name="osb", bufs=2))
        wpp = ctx.enter_context(tc.tile_pool(name="wpp", bufs=1))
        pss = ctx.enter_context(tc.tile_pool(name="pss", bufs=2, space="PSUM"))
        psy = ctx.enter_context(tc.tile_pool(name="psy", bufs=1, space="PSUM"))
        pfl = ctx.enter_context(tc.tile_pool(name="pfl", bufs=2, space="PSUM"))

        VA = [persist.tile([128, NH * 128], BF16, name=f"va{i}", tag=f"va{i}")
              for i in range(16)]
        YT = [persist.tile([128, T], BF16, name=f"yt{i}", tag=f"yt{i}")
              for i in range(4)]
        MSK = persist.tile([128, 256], BF16, name="msk", tag="msk")
        ones_f32 = persist.tile([128, 64], F32, name="ones_f32", tag="ones_f32")
        ones64 = persist.tile([1, 64], F32R, name="ones64", tag="ones64")
        nc.vector.memset(ones_f32, 1.0)
        nc.vector.tensor_copy(ones64, ones_f32[0:1, :])

        # ---- startup DMAs, triggers spread across engine queues ----
        # sync: wv (needed first), then mask + wp
        WV = []
        for c in range(8):
            w = wsl.tile([128, G], BF16, name=f"w{c}", tag=f"w{c}")
            nc.sync.dma_start(out=w, in_=wv[c * 128 : (c + 1) * 128, :])
            WV.append(w)
        # gpsimd: xT halves (2KB lines), half1 for all c then half2
        XT = []
        for c in range(8):
            t = xw.tile([128, T], BF16, name=f"x{c}", tag=f"x{c}")
            XT.append(t)
        for c in range(8):
            nc.gpsimd.dma_start(
                out=XT[c][:, 0 : T // 2],
                in_=xT[c * 128 : (c + 1) * 128, 0 : T // 2],
            )
        for c in range(8):
            nc.gpsimd.dma_start(
                out=XT[c][:, T // 2 : T],
                in_=xT[c * 128 : (c + 1) * 128, T // 2 : T],
            )
        nc.sync.dma_start(out=MSK, in_=mask[:, :])
        WP = []
        for cb in range(4):
            w = wpp.tile([128, C], BF16, name=f"wpj{cb}", tag=f"wpj{cb}")
            nc.sync.dma_start(out=w, in_=wp[cb * 128 : (cb + 1) * 128, :])
            WP.append(w)

        # V-augmentation ones columns
        ones_col = ones_f32[:, 0:8].rearrange("p (h o) -> p h o", o=1)
        for tb in range(16):
            vdst = VA[tb].rearrange("p (h e) -> p h e", e=128)[:, :, 64:65]
            nc.vector.tensor_copy(vdst, ones_col)

        # ---------------- V units ----------------
        def make_v_unit(tb):
            def unit():
                ps = pfl.tile([128, 512], F32, name="fill", tag="fill")
                for c in range(8):
                    nc.tensor.matmul(
                        ps,
                        XT[c][:, tb * 128 : (tb + 1) * 128],
                        WV[c],
                        start=(c == 0),
                        stop=(c == 7),
                    )
                vdst = VA[tb].rearrange("p (h e) -> p h e", e=128)[:, :, 0:64]
                nc.vector.tensor_copy(
                    vdst, ps.rearrange("p (h d) -> p h d", d=64)
                )
            return unit

        v_units = [make_v_unit(tb) for tb in range(16)]

        # ---------------- QK machinery ----------------
        def emit_w_slices(hp):
            # hp0 slices triggered from the (idle) scalar queue at startup;
            # later head-pairs from sync mid-kernel.
            eng = nc.scalar if hp == 0 else nc.sync
            tiles = {}
            for mat, dram in (("q", wq), ("k", wk)):
                lst = []
                for c in range(8):
                    w = wqk.tile(
                        [128, 128], BF16, name=f"w{mat}{c}", tag=f"w{mat}{c}"
                    )
                    eng.dma_start(
                        out=w,
                        in_=dram[
                            c * 128 : (c + 1) * 128,
                            hp * 128 : (hp + 1) * 128,
                        ],
                    )
                    lst.append(w)
                tiles[mat] = lst
            return tiles

        def make_qk_units(hp):
            wtiles = emit_w_slices(hp)
            qt = qtkt.tile([128, T], BF16, name="qtP", tag="qtP")
            kt = qtkt.tile([128, T], BF16, name="ktP", tag="ktP")
            units = {}
            for mat, dst in (("q", qt), ("k", kt)):
                for t4 in range(4):
                    def unit(mat=mat, dst=dst, t4=t4):
                        ps = pfl.tile([128, 512], F32, name="fill", tag="fill")
                        for c in range(8):
                            nc.tensor.matmul(
                                ps,
                                wtiles[mat][c],
                                XT[c][:, t4 * 512 : (t4 + 1) * 512],
                                start=(c == 0),
                                stop=(c == 7),
                            )
                        nc.vector.tensor_copy(
                            dst[:, t4 * 512 : (t4 + 1) * 512], ps
                        )
                    units[(mat, t4)] = unit
            return qt, kt, units

        # ---------- proj units (queued once YT token-cols are final) ----------
        def proj_units(tb):
            ot = {}
            def unit_ch(ch):
                def unit():
                    if ch == 0:
                        ot["t"] = osb.tile([128, C], BF16, name="ot", tag="ot")
                    ps = pfl.tile([128, 512], F32, name="fill", tag="fill")
                    for cb in range(4):
                        nc.tensor.matmul(
                            ps,
                            YT[cb][:, tb * 128 : (tb + 1) * 128],
                            WP[cb][:, ch * 512 : (ch + 1) * 512],
                            start=(cb == 0),
                            stop=(cb == 3),
                        )
                    nc.vector.tensor_copy(
                        ot["t"][:, ch * 512 : (ch + 1) * 512], ps
                    )
                    if ch == 1:
                        nc.sync.dma_start(
                            out=out[tb * 128 : (tb + 1) * 128, :], in_=ot["t"]
                        )
                return unit
            return [unit_ch(0), unit_ch(1)]

        def tail_units(qc):
            units = []
            for tb in range(qc * 4, qc * 4 + 4):
                units.extend(proj_units(tb))
            return units

        # ---------------- attention ----------------
        fill_q = deque()

        def pump(n):
            for _ in range(min(n, len(fill_q))):
                fill_q.popleft()()

        pend = deque()  # global AV deferral queue: (emit_fn, post_fn|None)

        def pop_av():
            emit, post = pend.popleft()
            emit()
            if post is not None:
                post()

        def push_av(emit, post=None):
            pend.append((emit, post))
            if len(pend) > DEPTH:
                pop_av()

        def attention(hp, qt, kt, qc):
            q0 = qc * QCH
            nkb = (qc + 1) * 4
            hA, hB = 2 * hp, 2 * hp + 1
            ytA = psy.tile([128, QCH], F32, name="ytA", tag="ytA")
            ytB = psy.tile([128, QCH], F32, name="ytB", tag="ytB")

            def emit_av(kb, pt, off, w):
                def go():
                    nc.tensor.matmul(
                        ytA[:, off : off + w],
                        VA[kb][:, hA * 128 : hA * 128 + 128],
                        pt[:, off : off + w],
                        start=(kb == 0),
                        stop=(kb == nkb - 1),
                    )
                    nc.tensor.matmul(
                        ytB[:, off : off + w],
                        VA[kb][:, hB * 128 : hB * 128 + 128],
                        pt[:, 512 + off : 512 + off + w],
                        start=(kb == 0),
                        stop=(kb == nkb - 1),
                    )
                return go

            def finalize():
                for sub, yt in ((0, ytA), (1, ytB)):
                    ysl = YT[hp][sub * 64 : (sub + 1) * 64, q0 : q0 + QCH]
                    nc.vector.tensor_copy(ysl, yt[0:64, :])
                    lf = nrm.tile([1, 512], F32, name="lf", tag="lf")
                    nc.vector.tensor_copy(lf, yt[64:65, :])
                    lf2 = nrm.tile([1, 512], F32, name="lf2", tag="lf2")
                    nc.vector.reciprocal_approx_fast(lf2, lf)
                    lr = lrp.tile([1, 512], F32R, name="lr", tag="lr")
                    nc.vector.tensor_copy(lr, lf2)
                    rb = pfl.tile([64, 512], F32, name="fill", tag="fill")
                    nc.tensor.matmul(rb, ones64, lr, start=True, stop=True)
                    nc.vector.tensor_mul(ysl, ysl, rb)
                if hp == 3:
                    fill_q.extend(tail_units(qc))

            for kb in range(nkb):
                j = kb - qc * 4
                off = j * 128 if j >= 1 else 0
                w = 512 - off
                ksl = slice(kb * KBLK, (kb + 1) * KBLK)
                sAB = pss.tile([128, 1024], F32, name="sAB", tag="sAB")
                nc.tensor.matmul(
                    sAB[:, off : 512],
                    kt[0:64, ksl],
                    qt[0:64, q0 + off : q0 + QCH],
                    start=True,
                    stop=True,
                    tile_position=(0, 0),
                )
                nc.tensor.matmul(
                    sAB[:, 512 + off : 1024],
                    kt[64:128, ksl],
                    qt[64:128, q0 + off : q0 + QCH],
                    start=True,
                    stop=True,
                    tile_position=(64, 0),
                )
                pt = ptp.tile([128, 1024], BF16, name="pt", tag="pt")
                if j >= 1:
                    sview = sAB.rearrange("p (s q) -> p s q", s=2)[:, :, off:512]
                    pview = pt.rearrange("p (s q) -> p s q", s=2)[:, :, off:512]
                    nc.scalar.activation(pview, sview, EXP, scale=0.125)
                else:
                    nc.scalar.activation(pt, sAB, EXP, scale=0.125)
                if j >= 0:
                    pv = pt.rearrange("p (s q) -> p s q", s=2)[
                        :, :, off : off + 128
                    ]
                    nc.vector.tensor_mul(
                        pv, pv, MSK.rearrange("p (s q) -> p s q", s=2)
                    )
                if hp in (0, 3) or kb % 4 == 1:
                    pump(1)
                push_av(
                    emit_av(kb, pt, off, w),
                    finalize if kb == nkb - 1 else None,
                )

        # ---------------- main schedule ----------------
        # eager: V token-blocks 0-3 and the first qt/kt chunk
        for tb in range(4):
            v_units[tb]()
        qt, kt, qk0 = make_qk_units(0)
        qk0[("k", 0)]()
        qk0[("q", 0)]()
        # remaining V blocks + qt/kt chunks as just-in-time fillers
        for t4 in range(1, 4):
            fill_q.append(qk0[("k", t4)])
            fill_q.append(qk0[("q", t4)])
            fill_q.extend(v_units[4 * t4 : 4 * t4 + 4])

        for hp in range(4):
            nqt = nkt = None
            if hp < 3:
                nqt, nkt, nunits = make_qk_units(hp + 1)
                for t4 in range(4):
                    fill_q.append(nunits[("q", t4)])
                    fill_q.append(nunits[("k", t4)])
            for qc in range(4):
                attention(hp, qt, kt, qc)
            # qk units of hp+1 must be fully emitted before its S reads them
            pump(len(fill_q))
            if hp < 3:
                qt, kt = nqt, nkt
        while pend:
            pop_av()
        pump(len(fill_q))

    nc.compile()
    return nc


_NC_CACHE = None


def kernel(x0, w_attn, w_proj, _trace=False, _tmpdir=None):
    global _NC_CACHE
    import ml_dtypes

    from concourse.bass_utils import run_bass_kernel_spmd

    BF = ml_dtypes.bfloat16
    x0 = np.asarray(x0, dtype=np.float32)
    w_attn = np.asarray(w_attn, dtype=np.float32)
    w_proj = np.asarray(w_proj, dtype=np.float32)
    B = x0.shape[0]

    if _NC_CACHE is None:
        _NC_CACHE = _build_nc()
    nc = _NC_CACHE

    tri = np.triu(np.ones((128, 128), dtype=np.float32))
    msk = np.concatenate([tri, tri], axis=1).astype(BF)
    in_maps = []
    for core in range(8):
        b, g = divmod(core, 2)
        in_maps.append(
            {
                "xT": np.ascontiguousarray(x0[b].T).astype(BF),
                "wq": np.ascontiguousarray(
                    w_attn[:, g * G : (g + 1) * G]
                ).astype(BF),
                "wk": np.ascontiguousarray(
                    w_attn[:, C + g * G : C + (g + 1) * G]
                ).astype(BF),
                "wv": np.ascontiguousarray(
                    w_attn[:, 2 * C + g * G : 2 * C + (g + 1) * G]
                ).astype(BF),
                "wp": np.ascontiguousarray(
                    w_proj[g * G : (g + 1) * G, :]
                ).astype(BF),
                "mask": msk,
            }
        )

    res = run_bass_kernel_spmd(
        nc, in_maps, list(range(8)), trace=_trace, tmpdir=_tmpdir
    )
    outp = np.empty((B, T, C), dtype=np.float32)
    for b in range(B):
        outp[b] = np.asarray(
            res.results[2 * b]["out"], dtype=np.float32
        ) + np.asarray(res.results[2 * b + 1]["out"], dtype=np.float32)
    if _trace:
        kernel.last_exec_time_ns = res.exec_time_ns
    return outp


# revision 14
# speedup vs baseline: 1.0460x; 1.0460x over previous
"""Causal self-attention (B=4, T=2048, C=1024, H=16) on 8 trn2 NeuronCores.

Sharding: core = (batch b, head-group g), b in 0..3, g in 0..1. Each core does
8 heads of one batch element (Megatron column split of w_attn, row split of
w_proj); host sums the two partial projection outputs per batch element.

Per-core kernel, v4 (eager-start, globally software-pipelined, PE kept dense):
 - All DRAM inputs bf16; output bf16 (host upcasts and sums partials).
 - Q^T,K^T computed transposed (lhsT=W-block, rhs=x^T-block) so attention
   needs no transposes; V natural with a ones column per head so the
   attention AV matmul accumulates the softmax denominator l for free.
 - Eager start: only V token-blocks 0-3 and the first qt/kt chunk are
   computed before attention begins; the remaining V blocks and qt/kt
   chunks become PE filler units pumped just-in-time inside the
   ACT-bound attention loop (measured exp rate ~1.16 ns/elem makes the
   steady state a knife-edge PE/ACT balance, so fillers are spread
   evenly: every k-block for hp 0/3, every 4th for hp 1/2).
 - Attention per head-pair: S^T for both heads row-tiled into one
   [128,1024] PSUM tile per k-block; one exp per k-block (3D AP covers
   both heads, scale=1/8 folded in); causal mask only on diagonal
   blocks via one doubled-mask bf16 multiply; AV deferred DEPTH k-blocks
   through a GLOBAL queue crossing qc/head-pair boundaries so the
   S->exp->AV pipeline never drains mid-kernel (keeps PE p-state at max
   clock).
 - qc finalize inline (attached to the last deferred AV): Y^T copy on
   the scalar engine (Copy activation), reciprocal straight from PSUM
   on vector, rank-1 broadcast matmul + in-place multiply immediately;
   output projection for the finished token blocks queued as filler.
 - Startup: DMA triggers spread across engine queues (sync: wv/mask/wp,
   gpsimd: x halves, scalar: hp0 qk weights) so descriptor writes don't
   serialize.
"""

import sys

if "/opt/trn_rl_repo" not in sys.path:
    sys.path.insert(0, "/opt/trn_rl_repo")

import numpy as np

T = 2048
C = 1024
G = 512          # per-core head-group width (8 heads x 64)
D = 64           # head dim
NH = 8           # heads per core
QCH = 512        # query chunk
KBLK = 128       # key block
DEPTH = 3        # AV deferral depth in k-blocks (global queue)


def _build_nc():
    from collections import deque
    from contextlib import ExitStack

    import concourse.bass as bass
    import concourse.mybir as mybir
    import concourse.tile as tile
    from concourse import bacc

    F32 = mybir.dt.float32
    F32R = mybir.dt.float32r
    BF16 = mybir.dt.bfloat16
    EXP = mybir.ActivationFunctionType.Exp
    CPY = mybir.ActivationFunctionType.Copy

    nc = bacc.Bacc("TRN2", target_bir_lowering=False)

    xT = nc.dram_tensor("xT", [C, T], BF16, kind="ExternalInput")
    wq = nc.dram_tensor("wq", [C, G], BF16, kind="ExternalInput")
    wk = nc.dram_tensor("wk", [C, G], BF16, kind="ExternalInput")
    wv = nc.dram_tensor("wv", [C, G], BF16, kind="ExternalInput")
    wp = nc.dram_tensor("wp", [G, C], BF16, kind="ExternalInput")
    mask = nc.dram_tensor("mask", [128, 256], BF16, kind="ExternalInput")
    out = nc.dram_tensor("out", [T, C], BF16, kind="ExternalOutput")

    with tile.TileContext(nc) as tc, ExitStack() as ctx:
        persist = ctx.enter_context(tc.tile_pool(name="persist", bufs=1))
        xw = ctx.enter_context(tc.tile_pool(name="xw", bufs=1))
        wsl = ctx.enter_context(tc.tile_pool(name="wsl", bufs=1))
        wqk = ctx.enter_context(tc.tile_pool(name="wqk", bufs=2))
        qtkt = ctx.enter_context(tc.tile_pool(name="qtkt", bufs=2))
        ptp = ctx.enter_context(tc.tile_pool(name="ptp", bufs=DEPTH + 1))
        nrm = ctx.enter_context(tc.tile_pool(name="nrm", bufs=2))
        lrp = ctx.enter_context(tc.tile_pool(name="lrp", bufs=12))
        osb = ctx.enter_context(tc.tile_pool(name="osb", bufs=2))
        wpp = ctx.enter_context(tc.tile_pool(name="wpp", bufs=1))
        pss = ctx.enter_context(tc.tile_pool(name="pss", bufs=2, space="PSUM"))
        psy = ctx.enter_context(tc.tile_pool(name="psy", bufs=1, space="PSUM"))
        pfl = ctx.enter_context(tc.tile_pool(name="pfl", bufs=2, space="PSUM"))

        VA = [persist.tile([128, NH * 128], BF16, name=f"va{i}", tag=f"va{i}")
              for i in range(16)]
        YT = [persist.tile([128, T], BF16, name=f"yt{i}", tag=f"yt{i}")
              for i in range(4)]
        MSK = persist.tile([128, 256], BF16, name="msk", tag="msk")
        ones_f32 = persist.tile([128, 64], F32, name="ones_f32", tag="ones_f32")
        ones64 = persist.tile([1, 64], F32R, name="ones64", tag="ones64")
        nc.vector.memset(ones_f32, 1.0)
        nc.vector.tensor_copy(ones64, ones_f32[0:1, :])

        # ---- startup DMAs, triggers spread across engine queues ----
        # sync: wv (needed first), then mask + wp
        WV = []
        for c in range(8):
            w = wsl.tile([128, G], BF16, name=f"w{c}", tag=f"w{c}")
            nc.sync.dma_start(out=w, in_=wv[c * 128 : (c + 1) * 128, :])
            WV.append(w)
        # gpsimd: xT halves (2KB lines), half1 for all c then half2
        XT = []
        for c in range(8):
            t = xw.tile([128, T], BF16, name=f"x{c}", tag=f"x{c}")
            XT.append(t)
        for c in range(8):
            nc.gpsimd.dma_start(
                out=XT[c][:, 0 : T // 2],
                in_=xT[c * 128 : (c + 1) * 128, 0 : T // 2],
            )
        for c in range(8):
            nc.gpsimd.dma_start(
                out=XT[c][:, T // 2 : T],
                in_=xT[c * 128 : (c + 1) * 128, T // 2 : T],
            )
        nc.sync.dma_start(out=MSK, in_=mask[:, :])
        WP = []
        for cb in range(4):
            w = wpp.tile([128, C], BF16, name=f"wpj{cb}", tag=f"wpj{cb}")
            nc.sync.dma_start(out=w, in_=wp[cb * 128 : (cb + 1) * 128, :])
            WP.append(w)

        # V-augmentation ones columns
        ones_col = ones_f32[:, 0:8].rearrange("p (h o) -> p h o", o=1)
        for tb in range(16):
            vdst = VA[tb].rearrange("p (h e) -> p h e", e=128)[:, :, 64:65]
            nc.vector.tensor_copy(vdst, ones_col)

        # ---------------- V units ----------------
        def make_v_unit(tb):
            def unit():
                ps = pfl.tile([128, 512], F32, name="fill", tag="fill")
                for c in range(8):
                    nc.tensor.matmul(
                        ps,
                        XT[c][:, tb * 128 : (tb + 1) * 128],
                        WV[c],
                        start=(c == 0),
                        stop=(c == 7),
                    )
                vdst = VA[tb].rearrange("p (h e) -> p h e", e=128)[:, :, 0:64]
                nc.vector.tensor_copy(
                    vdst, ps.rearrange("p (h d) -> p h d", d=64)
                )
            return unit

        v_units = [make_v_unit(tb) for tb in range(16)]

        # ---------------- QK machinery ----------------
        def emit_w_slices(hp):
            # hp0 slices triggered from the (idle) scalar queue at startup;
            # later head-pairs from sync mid-kernel.
            eng = nc.scalar if hp == 0 else nc.sync
            tiles = {}
            for mat, dram in (("q", wq), ("k", wk)):
                lst = []
                for c in range(8):
                    w = wqk.tile(
                        [128, 128], BF16, name=f"w{mat}{c}", tag=f"w{mat}{c}"
                    )
                    eng.dma_start(
                        out=w,
                        in_=dram[
                            c * 128 : (c + 1) * 128,
                            hp * 128 : (hp + 1) * 128,
                        ],
                    )
                    lst.append(w)
                tiles[mat] = lst
            return tiles

        def make_qk_units(hp):
            wtiles = emit_w_slices(hp)
            qt = qtkt.tile([128, T], BF16, name="qtP", tag="qtP")
            kt = qtkt.tile([128, T], BF16, name="ktP", tag="ktP")
            units = {}
            for mat, dst in (("q", qt), ("k", kt)):
                for t4 in range(4):
                    def unit(mat=mat, dst=dst, t4=t4):
                        ps = pfl.tile([128, 512], F32, name="fill", tag="fill")
                        for c in range(8):
                            nc.tensor.matmul(
                                ps,
                                wtiles[mat][c],
                                XT[c][:, t4 * 512 : (t4 + 1) * 512],
                                start=(c == 0),
                                stop=(c == 7),
                            )
                        nc.vector.tensor_copy(
                            dst[:, t4 * 512 : (t4 + 1) * 512], ps
                        )
                    units[(mat, t4)] = unit
            return qt, kt, units

        # ---------- proj units (queued once YT token-cols are final) ----------
        def proj_units(tb):
            ot = {}
            def unit_ch(ch):
                def unit():
                    if ch == 0:
                        ot["t"] = osb.tile([128, C], BF16, name="ot", tag="ot")
                    ps = pfl.tile([128, 512], F32, name="fill", tag="fill")
                    for cb in range(4):
                        nc.tensor.matmul(
                            ps,
                            YT[cb][:, tb * 128 : (tb + 1) * 128],
                            WP[cb][:, ch * 512 : (ch + 1) * 512],
                            start=(cb == 0),
                            stop=(cb == 3),
                        )
                    nc.vector.tensor_copy(
                        ot["t"][:, ch * 512 : (ch + 1) * 512], ps
                    )
                    if ch == 1:
                        nc.sync.dma_start(
                            out=out[tb * 128 : (tb + 1) * 128, :], in_=ot["t"]
                        )
                return unit
            return [unit_ch(0), unit_ch(1)]

        def tail_units(qc):
            units = []
            for tb in range(qc * 4, qc * 4 + 4):
                units.extend(proj_units(tb))
            return units

        # ---------------- attention ----------------
        fill_q = deque()

        def pump(n):
            for _ in range(min(n, len(fill_q))):
                fill_q.popleft()()

        pend = deque()  # global AV deferral queue: (emit_fn, post_fn|None)

        def pop_av():
            emit, post = pend.popleft()
            emit()
            if post is not None:
                post()

        def push_av(emit, post=None):
            pend.append((emit, post))
            if len(pend) > DEPTH:
                pop_av()

        def attention(hp, qt, kt, qc):
            q0 = qc * QCH
            nkb = (qc + 1) * 4
            hA, hB = 2 * hp, 2 * hp + 1
            ytA = psy.tile([128, QCH], F32, name="ytA", tag="ytA")
            ytB = psy.tile([128, QCH], F32, name="ytB", tag="ytB")

            def emit_av(kb, pt, off, w):
                def go():
                    nc.tensor.matmul(
                        ytA[:, off : off + w],
                        VA[kb][:, hA * 128 : hA * 128 + 128],
                        pt[:, off : off + w],
                        start=(kb == 0),
                        stop=(kb == nkb - 1),
                    )
                    nc.tensor.matmul(
                        ytB[:, off : off + w],
                        VA[kb][:, hB * 128 : hB * 128 + 128],
                        pt[:, 512 + off : 512 + off + w],
                        start=(kb == 0),
                        stop=(kb == nkb - 1),
                    )
                return go

            def finalize():
                for sub, yt in ((0, ytA), (1, ytB)):
                    ysl = YT[hp][sub * 64 : (sub + 1) * 64, q0 : q0 + QCH]
                    nc.vector.tensor_copy(ysl, yt[0:64, :])
                    lf = nrm.tile([1, 512], F32, name="lf", tag="lf")
                    nc.vector.tensor_copy(lf, yt[64:65, :])
                    lf2 = nrm.tile([1, 512], F32, name="lf2", tag="lf2")
                    nc.vector.reciprocal_approx_fast(lf2, lf)
                    lr = lrp.tile([1, 512], F32R, name="lr", tag="lr")
                    nc.vector.tensor_copy(lr, lf2)

                    def norm_fin(ysl=ysl, lr=lr):
                        rb = pfl.tile([64, 512], F32, name="fill", tag="fill")
                        nc.tensor.matmul(rb, ones64, lr, start=True, stop=True)
                        nc.vector.tensor_mul(ysl, ysl, rb)
                    fill_q.append(norm_fin)
                # tails must be queued AFTER this qc's norm muls (FIFO): the
                # projection reads the very YT columns the norms finalize.
                if hp == 3:
                    fill_q.extend(tail_units(qc))

            for kb in range(nkb):
                j = kb - qc * 4
                off = j * 128 if j >= 1 else 0
                w = 512 - off
                ksl = slice(kb * KBLK, (kb + 1) * KBLK)
                sAB = pss.tile([128, 1024], F32, name="sAB", tag="sAB")
                nc.tensor.matmul(
                    sAB[:, off : 512],
                    kt[0:64, ksl],
                    qt[0:64, q0 + off : q0 + QCH],
                    start=True,
                    stop=True,
                    tile_position=(0, 0),
                )
                nc.tensor.matmul(
                    sAB[:, 512 + off : 1024],
                    kt[64:128, ksl],
                    qt[64:128, q0 + off : q0 + QCH],
                    start=True,
                    stop=True,
                    tile_position=(64, 0),
                )
                pt = ptp.tile([128, 1024], BF16, name="pt", tag="pt")
                if j >= 1:
                    sview = sAB.rearrange("p (s q) -> p s q", s=2)[:, :, off:512]
                    pview = pt.rearrange("p (s q) -> p s q", s=2)[:, :, off:512]
                    nc.scalar.activation(pview, sview, EXP, scale=0.125)
                else:
                    nc.scalar.activation(pt, sAB, EXP, scale=0.125)
                if j >= 0:
                    pv = pt.rearrange("p (s q) -> p s q", s=2)[
                        :, :, off : off + 128
                    ]
                    nc.vector.tensor_mul(
                        pv, pv, MSK.rearrange("p (s q) -> p s q", s=2)
                    )
                if hp in (0, 3) or kb % 2 == 1:
                    pump(1)
                push_av(
                    emit_av(kb, pt, off, w),
                    finalize if kb == nkb - 1 else None,
                )

        # ---------------- main schedule ----------------
        for tb in range(16):
            v_units[tb]()
        qt, kt, qk0 = make_qk_units(0)
        qk0[("q", 0)]()
        qk0[("k", 0)]()
        for t4 in range(1, 4):
            fill_q.append(qk0[("q", t4)])
            fill_q.append(qk0[("k", t4)])

        for hp in range(4):
            nqt = nkt = None
            if hp < 3:
                nqt, nkt, nunits = make_qk_units(hp + 1)
                for t4 in range(4):
                    fill_q.append(nunits[("q", t4)])
                    fill_q.append(nunits[("k", t4)])
            for qc in range(4):
                attention(hp, qt, kt, qc)
                if hp in (0, 3):
                    pump(2)
            # qk units of hp+1 must be fully emitted before its S reads them
            pump(len(fill_q))
            if hp < 3:
                qt, kt = nqt, nkt
        while pend:
            pop_av()
        pump(len(fill_q))

    nc.compile()
    return nc


_NC_CACHE = None


def kernel(x0, w_attn, w_proj, _trace=False, _tmpdir=None):
    global _NC_CACHE
    import ml_dtypes

    from concourse.bass_utils import run_bass_kernel_spmd

    BF = ml_dtypes.bfloat16
    x0 = np.asarray(x0, dtype=np.float32)
    w_attn = np.asarray(w_attn, dtype=np.float32)
    w_proj = np.asarray(w_proj, dtype=np.float32)
    B = x0.shape[0]

    if _NC_CACHE is None:
        _NC_CACHE = _build_nc()
    nc = _NC_CACHE

    tri = np.triu(np.ones((128, 128), dtype=np.float32))
    msk = np.concatenate([tri, tri], axis=1).astype(BF)
    in_maps = []
    for core in range(8):
        b, g = divmod(core, 2)
        in_maps.append(
            {
                "xT": np.ascontiguousarray(x0[b].T).astype(BF),
                "wq": np.ascontiguousarray(
                    w_attn[:, g * G : (g + 1) * G]
                ).astype(BF),
                "wk": np.ascontiguousarray(
                    w_attn[:, C + g * G : C + (g + 1) * G]
                ).astype(BF),
                "wv": np.ascontiguousarray(
                    w_attn[:, 2 * C + g * G : 2 * C + (g + 1) * G]
                ).astype(BF),
                "wp": np.ascontiguousarray(
                    w_proj[g * G : (g + 1) * G, :]
                ).astype(BF),
                "mask": msk,
            }
        )

    res = run_bass_kernel_spmd(
        nc, in_maps, list(range(8)), trace=_trace, tmpdir=_tmpdir
    )
    outp = np.empty((B, T, C), dtype=np.float32)
    for b in range(B):
        outp[b] = np.asarray(
            res.results[2 * b]["out"], dtype=np.float32
        ) + np.asarray(res.results[2 * b + 1]["out"], dtype=np.float32)
    if _trace:
        kernel.last_exec_time_ns = res.exec_time_ns
    return outp


# revision 23
# speedup vs baseline: 1.0490x; 1.0029x over previous
"""Causal self-attention (B=4, T=2048, C=1024, H=16) on 8 trn2 NeuronCores.

Sharding: core = (batch b, head-group g), b in 0..3, g in 0..1. Each core does
8 heads of one batch element (Megatron column split of w_attn, row split of
w_proj); host sums the two partial projection outputs per batch element.

Per-core kernel, v4 (eager-start, globally software-pipelined, PE kept dense):
 - All DRAM inputs bf16; output bf16 (host upcasts and sums partials).
 - Q^T,K^T computed transposed (lhsT=W-block, rhs=x^T-block) so attention
   needs no transposes; V natural with a ones column per head so the
   attention AV matmul accumulates the softmax denominator l for free.
 - Eager start: only V token-blocks 0-3 and the first qt/kt chunk are
   computed before attention begins; the remaining V blocks and qt/kt
   chunks become PE filler units pumped just-in-time inside the
   ACT-bound attention loop (measured exp rate ~1.16 ns/elem makes the
   steady state a knife-edge PE/ACT balance, so fillers are spread
   evenly: every k-block for hp 0/3, every 4th for hp 1/2).
 - Attention per head-pair: S^T for both heads row-tiled into one
   [128,1024] PSUM tile per k-block; one exp per k-block (3D AP covers
   both heads, scale=1/8 folded in); causal mask only on diagonal
   blocks via one doubled-mask bf16 multiply; AV deferred DEPTH k-blocks
   through a GLOBAL queue crossing qc/head-pair boundaries so the
   S->exp->AV pipeline never drains mid-kernel (keeps PE p-state at max
   clock).
 - qc finalize inline (attached to the last deferred AV): Y^T copy on
   the scalar engine (Copy activation), reciprocal straight from PSUM
   on vector, rank-1 broadcast matmul + in-place multiply immediately;
   output projection for the finished token blocks queued as filler.
 - Startup: DMA triggers spread across engine queues (sync: wv/mask/wp,
   gpsimd: x halves, scalar: hp0 qk weights) so descriptor writes don't
   serialize.
"""

import sys

if "/opt/trn_rl_repo" not in sys.path:
    sys.path.insert(0, "/opt/trn_rl_repo")

import numpy as np

T = 2048
C = 1024
G = 512          # per-core head-group width (8 heads x 64)
D = 64           # head dim
NH = 8           # heads per core
QCH = 512        # query chunk
KBLK = 128       # key block
DEPTH = 3        # AV deferral depth in k-blocks (global queue)


def _build_nc():
    from collections import deque
    from contextlib import ExitStack

    import concourse.bass as bass
    import concourse.mybir as mybir
    import concourse.tile as tile
    from concourse import bacc

    F32 = mybir.dt.float32
    F32R = mybir.dt.float32r
    BF16 = mybir.dt.bfloat16
    EXP = mybir.ActivationFunctionType.Exp
    CPY = mybir.ActivationFunctionType.Copy

    nc = bacc.Bacc("TRN2", target_bir_lowering=False)

    xT = nc.dram_tensor("xT", [C, T], BF16, kind="ExternalInput")
    wq = nc.dram_tensor("wq", [C, G], BF16, kind="ExternalInput")
    wk = nc.dram_tensor("wk", [C, G], BF16, kind="ExternalInput")
    # host-packed: row p, col c*G+j = wv[c*128+p, j]; one contiguous DMA
    wv = nc.dram_tensor("wv", [128, 8 * G], BF16, kind="ExternalInput")
    wp = nc.dram_tensor("wp", [G, C], BF16, kind="ExternalInput")
    mask = nc.dram_tensor("mask", [128, 256], BF16, kind="ExternalInput")
    out = nc.dram_tensor("out", [T, C], BF16, kind="ExternalOutput")

    with tile.TileContext(nc) as tc, ExitStack() as ctx:
        persist = ctx.enter_context(tc.tile_pool(name="persist", bufs=1))
        xw = ctx.enter_context(tc.tile_pool(name="xw", bufs=1))
        wsl = ctx.enter_context(tc.tile_pool(name="wsl", bufs=1))
        wqk = ctx.enter_context(tc.tile_pool(name="wqk", bufs=2))
        qtkt = ctx.enter_context(tc.tile_pool(name="qtkt", bufs=2))
        ptp = ctx.enter_context(tc.tile_pool(name="ptp", bufs=DEPTH + 1))
        nrm = ctx.enter_context(tc.tile_pool(name="nrm", bufs=2))
        lrp = ctx.enter_context(tc.tile_pool(name="lrp", bufs=12))
        osb = ctx.enter_context(tc.tile_pool(name="osb", bufs=2))
        wpp = ctx.enter_context(tc.tile_pool(name="wpp", bufs=1))
        pss = ctx.enter_context(tc.tile_pool(name="pss", bufs=2, space="PSUM"))
        psy = ctx.enter_context(tc.tile_pool(name="psy", bufs=1, space="PSUM"))
        pfl = ctx.enter_context(tc.tile_pool(name="pfl", bufs=2, space="PSUM"))

        VA = [persist.tile([128, NH * 128], BF16, name=f"va{i}", tag=f"va{i}")
              for i in range(16)]
        YT = [persist.tile([128, T], BF16, name=f"yt{i}", tag=f"yt{i}")
              for i in range(4)]
        MSK = persist.tile([128, 256], BF16, name="msk", tag="msk")
        ones_f32 = persist.tile([128, 64], F32, name="ones_f32", tag="ones_f32")
        ones64 = persist.tile([1, 64], F32R, name="ones64", tag="ones64")
        nc.vector.memset(ones_f32, 1.0)
        nc.vector.tensor_copy(ones64, ones_f32[0:1, :])

        # ---- startup DMAs, triggers spread across engine queues ----
        # sync: wv in ONE contiguous 1MB transfer (host-packed), then mask + wp
        WVB = wsl.tile([128, 8 * G], BF16, name="wvb", tag="wvb")
        nc.sync.dma_start(out=WVB, in_=wv[:, :])
        WV = [WVB[:, c * G : (c + 1) * G] for c in range(8)]
        # gpsimd: xT first halves (2KB lines); scalar (after wqk0): second halves
        XT = []
        for c in range(8):
            t = xw.tile([128, T], BF16, name=f"x{c}", tag=f"x{c}")
            XT.append(t)
        for c in range(8):
            nc.gpsimd.dma_start(
                out=XT[c][:, 0 : T // 2],
                in_=xT[c * 128 : (c + 1) * 128, 0 : T // 2],
            )
        nc.sync.dma_start(out=MSK, in_=mask[:, :])
        WP = []
        for cb in range(4):
            w = wpp.tile([128, C], BF16, name=f"wpj{cb}", tag=f"wpj{cb}")
            nc.sync.dma_start(out=w, in_=wp[cb * 128 : (cb + 1) * 128, :])
            WP.append(w)

        # ---------------- QK weight slices ----------------
        def emit_w_slices(hp):
            # hp0 slices triggered from the (idle) scalar queue at startup;
            # later head-pairs from sync mid-kernel.
            eng = nc.scalar if hp == 0 else nc.sync
            tiles = {}
            for mat, dram in (("q", wq), ("k", wk)):
                lst = []
                for c in range(8):
                    w = wqk.tile(
                        [128, 128], BF16, name=f"w{mat}{c}", tag=f"w{mat}{c}"
                    )
                    eng.dma_start(
                        out=w,
                        in_=dram[
                            c * 128 : (c + 1) * 128,
                            hp * 128 : (hp + 1) * 128,
                        ],
                    )
                    lst.append(w)
                tiles[mat] = lst
            return tiles

        wtiles0 = emit_w_slices(0)
        # scalar queue, after the hp0 qk slices: xT second halves
        for c in range(8):
            nc.scalar.dma_start(
                out=XT[c][:, T // 2 : T],
                in_=xT[c * 128 : (c + 1) * 128, T // 2 : T],
            )

        # V-augmentation ones columns
        ones_col = ones_f32[:, 0:8].rearrange("p (h o) -> p h o", o=1)
        for tb in range(16):
            vdst = VA[tb].rearrange("p (h e) -> p h e", e=128)[:, :, 64:65]
            nc.vector.tensor_copy(vdst, ones_col)

        # ---------------- V units ----------------
        def make_v_unit(tb):
            def unit():
                ps = pfl.tile([128, 512], F32, name="fill", tag="fill")
                for c in range(8):
                    nc.tensor.matmul(
                        ps,
                        XT[c][:, tb * 128 : (tb + 1) * 128],
                        WV[c],
                        start=(c == 0),
                        stop=(c == 7),
                    )
                vdst = VA[tb].rearrange("p (h e) -> p h e", e=128)[:, :, 0:64]
                nc.vector.tensor_copy(
                    vdst, ps.rearrange("p (h d) -> p h d", d=64)
                )
            return unit

        v_units = [make_v_unit(tb) for tb in range(16)]

        # ---------------- QK machinery ----------------
        def make_qk_units(hp, wtiles=None):
            if wtiles is None:
                wtiles = emit_w_slices(hp)
            qt = qtkt.tile([128, T], BF16, name="qtP", tag="qtP")
            kt = qtkt.tile([128, T], BF16, name="ktP", tag="ktP")
            units = {}
            for mat, dst in (("q", qt), ("k", kt)):
                for t4 in range(4):
                    def unit(mat=mat, dst=dst, t4=t4):
                        ps = pfl.tile([128, 512], F32, name="fill", tag="fill")
                        for c in range(8):
                            nc.tensor.matmul(
                                ps,
                                wtiles[mat][c],
                                XT[c][:, t4 * 512 : (t4 + 1) * 512],
                                start=(c == 0),
                                stop=(c == 7),
                            )
                        nc.vector.tensor_copy(
                            dst[:, t4 * 512 : (t4 + 1) * 512], ps
                        )
                    units[(mat, t4)] = unit
            return qt, kt, units

        # ---------- proj units (queued once YT token-cols are final) ----------
        def proj_units(tb):
            ot = {}
            def unit_ch(ch):
                def unit():
                    if ch == 0:
                        ot["t"] = osb.tile([128, C], BF16, name="ot", tag="ot")
                    ps = pfl.tile([128, 512], F32, name="fill", tag="fill")
                    for cb in range(4):
                        nc.tensor.matmul(
                            ps,
                            YT[cb][:, tb * 128 : (tb + 1) * 128],
                            WP[cb][:, ch * 512 : (ch + 1) * 512],
                            start=(cb == 0),
                            stop=(cb == 3),
                        )
                    nc.vector.tensor_copy(
                        ot["t"][:, ch * 512 : (ch + 1) * 512], ps
                    )
                    if ch == 1:
                        nc.sync.dma_start(
                            out=out[tb * 128 : (tb + 1) * 128, :], in_=ot["t"]
                        )
                return unit
            return [unit_ch(0), unit_ch(1)]

        def tail_units(qc):
            units = []
            for tb in range(qc * 4, qc * 4 + 4):
                units.extend(proj_units(tb))
            return units

        # ---------------- attention ----------------
        fill_q = deque()

        def pump(n):
            for _ in range(min(n, len(fill_q))):
                fill_q.popleft()()

        pend = deque()  # global AV deferral queue: (emit_fn, post_fn|None)

        def pop_av():
            emit, post = pend.popleft()
            emit()
            if post is not None:
                post()

        def attention(hp, qt, kt, qc):
            q0 = qc * QCH
            nkb = (qc + 1) * 4
            hA, hB = 2 * hp, 2 * hp + 1
            ytA = psy.tile([128, QCH], F32, name="ytA", tag="ytA")
            ytB = psy.tile([128, QCH], F32, name="ytB", tag="ytB")

            def emit_av(kb, pt, off, w):
                def go():
                    nc.tensor.matmul(
                        ytA[:, off : off + w],
                        VA[kb][:, hA * 128 : hA * 128 + 128],
                        pt[:, off : off + w],
                        start=(kb == 0),
                        stop=(kb == nkb - 1),
                    )
                    nc.tensor.matmul(
                        ytB[:, off : off + w],
                        VA[kb][:, hB * 128 : hB * 128 + 128],
                        pt[:, 512 + off : 512 + off + w],
                        start=(kb == 0),
                        stop=(kb == nkb - 1),
                    )
                return go

            def finalize():
                for sub, yt in ((0, ytA), (1, ytB)):
                    ysl = YT[hp][sub * 64 : (sub + 1) * 64, q0 : q0 + QCH]
                    nc.vector.tensor_copy(ysl, yt[0:64, :])
                    lf = nrm.tile([1, 512], F32, name="lf", tag="lf")
                    nc.vector.tensor_copy(lf, yt[64:65, :])
                    lf2 = nrm.tile([1, 512], F32, name="lf2", tag="lf2")
                    nc.vector.reciprocal_approx_fast(lf2, lf)
                    lr = lrp.tile([1, 512], F32R, name="lr", tag="lr")
                    nc.vector.tensor_copy(lr, lf2)

                    def norm_fin(ysl=ysl, lr=lr):
                        rb = pfl.tile([64, 512], F32, name="fill", tag="fill")
                        nc.tensor.matmul(rb, ones64, lr, start=True, stop=True)
                        nc.vector.tensor_mul(ysl, ysl, rb)
                    fill_q.append(norm_fin)
                # tails must be queued AFTER this qc's norm muls (FIFO): the
                # projection reads the very YT columns the norms finalize.
                if hp == 3:
                    fill_q.extend(tail_units(qc))

            for kb in range(nkb):
                j = kb - qc * 4
                off = j * 128 if j >= 1 else 0
                w = 512 - off
                ksl = slice(kb * KBLK, (kb + 1) * KBLK)
                # emit ready work (filler + deferred AV) BEFORE this S-pair:
                # the PE queue is in-order and the S-pair WAR-waits on the
                # exp two k-blocks back, so ready work must precede it.
                if hp in (0, 3) or kb % 2 == 1:
                    pump(1)
                if len(pend) >= DEPTH:
                    pop_av()
                sAB = pss.tile([128, 1024], F32, name="sAB", tag="sAB")
                nc.tensor.matmul(
                    sAB[:, off : 512],
                    kt[0:64, ksl],
                    qt[0:64, q0 + off : q0 + QCH],
                    start=True,
                    stop=True,
                    tile_position=(0, 0),
                )
                nc.tensor.matmul(
                    sAB[:, 512 + off : 1024],
                    kt[64:128, ksl],
                    qt[64:128, q0 + off : q0 + QCH],
                    start=True,
                    stop=True,
                    tile_position=(64, 0),
                )
                pt = ptp.tile([128, 1024], BF16, name="pt", tag="pt")
                if j >= 1:
                    sview = sAB.rearrange("p (s q) -> p s q", s=2)[:, :, off:512]
                    pview = pt.rearrange("p (s q) -> p s q", s=2)[:, :, off:512]
                    nc.scalar.activation(pview, sview, EXP, scale=0.125)
                else:
                    nc.scalar.activation(pt, sAB, EXP, scale=0.125)
                if j >= 0:
                    pv = pt.rearrange("p (s q) -> p s q", s=2)[
                        :, :, off : off + 128
                    ]
                    nc.vector.tensor_mul(
                        pv, pv, MSK.rearrange("p (s q) -> p s q", s=2)
                    )
                pend.append(
                    (emit_av(kb, pt, off, w),
                     finalize if kb == nkb - 1 else None)
                )

        # ---------------- main schedule ----------------
        for tb in range(16):
            v_units[tb]()
        qt, kt, qk0 = make_qk_units(0, wtiles0)
        qk0[("q", 0)]()
        qk0[("k", 0)]()
        for t4 in range(1, 4):
            fill_q.append(qk0[("q", t4)])
            fill_q.append(qk0[("k", t4)])

        for hp in range(4):
            nqt = nkt = None
            if hp < 3:
                nqt, nkt, nunits = make_qk_units(hp + 1)
                for t4 in range(4):
                    fill_q.append(nunits[("q", t4)])
                    fill_q.append(nunits[("k", t4)])
            for qc in range(4):
                attention(hp, qt, kt, qc)
                if hp in (0, 3):
                    pump(2)
            # qk units of hp+1 must be fully emitted before its S reads them
            pump(len(fill_q))
            if hp < 3:
                qt, kt = nqt, nkt
        while pend:
            pop_av()
        pump(len(fill_q))

    nc.compile()
    return nc


_NC_CACHE = None


def kernel(x0, w_attn, w_proj, _trace=False, _tmpdir=None):
    global _NC_CACHE
    import ml_dtypes

    from concourse.bass_utils import run_bass_kernel_spmd

    BF = ml_dtypes.bfloat16
    x0 = np.asarray(x0, dtype=np.float32)
    w_attn = np.asarray(w_attn, dtype=np.float32)
    w_proj = np.asarray(w_proj, dtype=np.float32)
    B = x0.shape[0]

    if _NC_CACHE is None:
        _NC_CACHE = _build_nc()
    nc = _NC_CACHE

    tri = np.triu(np.ones((128, 128), dtype=np.float32))
    msk = np.concatenate([tri, tri], axis=1).astype(BF)
    in_maps = []
    for core in range(8):
        b, g = divmod(core, 2)
        in_maps.append(
            {
                "xT": np.ascontiguousarray(x0[b].T).astype(BF),
                "wq": np.ascontiguousarray(
                    w_attn[:, g * G : (g + 1) * G]
                ).astype(BF),
                "wk": np.ascontiguousarray(
                    w_attn[:, C + g * G : C + (g + 1) * G]
                ).astype(BF),
                "wv": np.ascontiguousarray(
                    w_attn[:, 2 * C + g * G : 2 * C + (g + 1) * G]
                    .reshape(8, 128, G)
                    .transpose(1, 0, 2)
                    .reshape(128, 8 * G)
                ).astype(BF),
                "wp": np.ascontiguousarray(
                    w_proj[g * G : (g + 1) * G, :]
                ).astype(BF),
                "mask": msk,
            }
        )

    res = run_bass_kernel_spmd(
        nc, in_maps, list(range(8)), trace=_trace, tmpdir=_tmpdir
    )
    outp = np.empty((B, T, C), dtype=np.float32)
    for b in range(B):
        outp[b] = np.asarray(
            res.results[2 * b]["out"], dtype=np.float32
        ) + np.asarray(res.results[2 * b + 1]["out"], dtype=np.float32)
    if _trace:
        kernel.last_exec_time_ns = res.exec_time_ns
    return outp
